# revision 2
# baseline (speedup 1.0000x reference)
"""Trainium2 Bass kernel for an AttentionBlock with a single KV token.

Math: with kv_len == 1 the softmax over the key axis is identically 1.0,
so the attention output for every query position equals v, and the
LayerNorm / q-projection never influence the output:

    kv      = cond_emb @ kv_w.T + kv_b          # (b, 2c)
    v_in    = kv[:, c:]                         # (b, c)
    v_full  = v_in @ wv.T + bv                  # (b, c)   wv = in_proj_w[2c:]
    av      = v_full @ out_w.T + out_b          # (b, c)
    y       = x + av[:, :, None, None]          # (b, c, h, w)

This is a tiny per-batch matmul chain plus one huge memory-bound
broadcast add.  Sharding: data-parallel over batch (8 batches/core),
weights replicated (host pre-transposed into matmul layouts).
"""

import numpy as np

import concourse.bacc as bacc
import concourse.mybir as mybir
from concourse.bass_utils import run_bass_kernel_spmd
from concourse.tile import TileContext

B, C, H, W = 64, 256, 64, 64
EMB = 512
HWD = H * W               # 4096
NCORES = 8
BS = B // NCORES          # 8 batches per core
ROWS = BS * C             # 2048 rows of length HW per core
NT = ROWS // 128          # 16 tiles of [128, 4096]
F32 = mybir.dt.float32

_CACHE = {}


def _build_nc():
    nc = bacc.Bacc("TRN2", target_bir_lowering=False, debug=False)

    x_d = nc.dram_tensor("x", [ROWS, HWD], F32, kind="ExternalInput").ap()
    # cond_sb layout: [p, e*8 + b] = cond_emb[b, 128e + p]
    cond_d = nc.dram_tensor("cond_t", [128, 4 * BS], F32, kind="ExternalInput").ap()
    # kvw_sb layout: [p, e*256 + j] = kv_w[256 + j, 128e + p]   (V-half, transposed)
    kvw_d = nc.dram_tensor("kvw_t", [128, 4 * C], F32, kind="ExternalInput").ap()
    # wv_sb layout:  [p, i*256 + j] = in_proj_w[512 + j, 128i + p]
    wv_d = nc.dram_tensor("wv_t", [128, 2 * C], F32, kind="ExternalInput").ap()
    # outw_sb layout: [p, j*256 + c] = out_w[c, 128j + p]
    outw_d = nc.dram_tensor("outw_t", [128, 2 * C], F32, kind="ExternalInput").ap()
    # bias layout: [p, u*3 + k]; k=0: kv_b[256+u*128+p], k=1: in_proj_b[512+u*128+p],
    # k=2: out_b[u*128+p]
    bias_d = nc.dram_tensor("bias", [128, 6], F32, kind="ExternalInput").ap()

    y_d = nc.dram_tensor("y", [ROWS, HWD], F32, kind="ExternalOutput").ap()

    with TileContext(nc) as tc:
        with (
            tc.tile_pool(name="const", bufs=1) as cpool,
            tc.tile_pool(name="psum", bufs=2, space="PSUM") as ppool,
            tc.tile_pool(name="small", bufs=2) as spool,
            tc.tile_pool(name="xio", bufs=8) as xpool,
        ):
            cond_sb = cpool.tile([128, 4 * BS], F32, tag="cond")
            kvw_sb = cpool.tile([128, 4 * C], F32, tag="kvw")
            wv_sb = cpool.tile([128, 2 * C], F32, tag="wv")
            outw_sb = cpool.tile([128, 2 * C], F32, tag="outw")
            bias_sb = cpool.tile([128, 6], F32, tag="bias")
            nc.sync.dma_start(out=cond_sb[:], in_=cond_d[:])
            nc.sync.dma_start(out=kvw_sb[:], in_=kvw_d[:])
            nc.sync.dma_start(out=wv_sb[:], in_=wv_d[:])
            nc.sync.dma_start(out=outw_sb[:], in_=outw_d[:])
            nc.sync.dma_start(out=bias_sb[:], in_=bias_d[:])

            # v_inT[u][p, b] = kv[b, 256 + u*128 + p]
            vin_sb = [spool.tile([128, BS], F32, tag=f"vin{u}", name=f"vin{u}") for u in range(2)]
            for u in range(2):
                pv = ppool.tile([128, BS], F32)
                for e in range(4):
                    nc.tensor.matmul(
                        out=pv[:],
                        lhsT=kvw_sb[:, e * C + u * 128 : e * C + u * 128 + 128],
                        rhs=cond_sb[:, e * BS : (e + 1) * BS],
                        start=(e == 0),
                        stop=(e == 3),
                    )
                nc.vector.tensor_scalar_add(
                    out=vin_sb[u][:], in0=pv[:], scalar1=bias_sb[:, 0 + u * 3 : 1 + u * 3]
                )

            # v_fullT[u][p, b] = v_full[b, u*128 + p]
            vf_sb = [spool.tile([128, BS], F32, tag=f"vf{u}", name=f"vf{u}") for u in range(2)]
            for u in range(2):
                pv = ppool.tile([128, BS], F32)
                for i in range(2):
                    nc.tensor.matmul(
                        out=pv[:],
                        lhsT=wv_sb[:, i * C + u * 128 : i * C + u * 128 + 128],
                        rhs=vin_sb[i][:],
                        start=(i == 0),
                        stop=(i == 1),
                    )
                nc.vector.tensor_scalar_add(
                    out=vf_sb[u][:], in0=pv[:], scalar1=bias_sb[:, 1 + u * 3 : 2 + u * 3]
                )

            # avT[u][p, b] = av[b, u*128 + p]
            av_sb = [spool.tile([128, BS], F32, tag=f"av{u}", name=f"av{u}") for u in range(2)]
            for u in range(2):
                pv = ppool.tile([128, BS], F32)
                for j in range(2):
                    nc.tensor.matmul(
                        out=pv[:],
                        lhsT=outw_sb[:, j * C + u * 128 : j * C + u * 128 + 128],
                        rhs=vf_sb[j][:],
                        start=(j == 0),
                        stop=(j == 1),
                    )
                nc.vector.tensor_scalar_add(
                    out=av_sb[u][:], in0=pv[:], scalar1=bias_sb[:, 2 + u * 3 : 3 + u * 3]
                )

            # Stream x: row r = b*256 + c ; tile t covers rows [128t, 128t+128)
            # -> batch b = t//2, channel c = (t%2)*128 + p, scalar = av_sb[t%2][p, t//2]
            for t in range(NT):
                u, b = t % 2, t // 2
                tile = xpool.tile([128, HWD], F32, tag="xt")
                nc.sync.dma_start(out=tile[:], in_=x_d[t * 128 : (t + 1) * 128, :])
                nc.vector.tensor_scalar_add(
                    out=tile[:], in0=tile[:], scalar1=av_sb[u][:, b : b + 1]
                )
                nc.scalar.dma_start(out=y_d[t * 128 : (t + 1) * 128, :], in_=tile[:])

    nc.compile()
    return nc


def _prep_consts(in_proj_w, in_proj_b, out_w, out_b, kv_w, kv_b):
    c = C
    # kvw_t: [p, e*256 + j] = kv_w[256 + j, 128e + p]
    kvw = np.ascontiguousarray(
        kv_w[c : 2 * c, :].T.reshape(4, 128, c).transpose(1, 0, 2).reshape(128, 4 * c)
    ).astype(np.float32)
    wv = np.ascontiguousarray(
        in_proj_w[2 * c :, :].T.reshape(2, 128, c).transpose(1, 0, 2).reshape(128, 2 * c)
    ).astype(np.float32)
    outw = np.ascontiguousarray(
        out_w.T.reshape(2, 128, c).transpose(1, 0, 2).reshape(128, 2 * c)
    ).astype(np.float32)
    bias = np.empty((128, 6), np.float32)
    for u in range(2):
        bias[:, u * 3 + 0] = kv_b[c + u * 128 : c + (u + 1) * 128]
        bias[:, u * 3 + 1] = in_proj_b[2 * c + u * 128 : 2 * c + (u + 1) * 128]
        bias[:, u * 3 + 2] = out_b[u * 128 : (u + 1) * 128]
    return kvw, wv, outw, bias


def make_in_maps(x, cond_emb, in_proj_w, in_proj_b, out_w, out_b, kv_w, kv_b):
    kvw, wv, outw, bias = _prep_consts(in_proj_w, in_proj_b, out_w, out_b, kv_w, kv_b)
    in_maps = []
    for r in range(NCORES):
        xs = np.ascontiguousarray(
            x[r * BS : (r + 1) * BS].reshape(ROWS, HWD), dtype=np.float32
        )
        ct = np.ascontiguousarray(
            cond_emb[r * BS : (r + 1) * BS]
            .T.reshape(4, 128, BS)
            .transpose(1, 0, 2)
            .reshape(128, 4 * BS)
        ).astype(np.float32)
        in_maps.append(
            {"x": xs, "cond_t": ct, "kvw_t": kvw, "wv_t": wv, "outw_t": outw, "bias": bias}
        )
    return in_maps


def get_nc():
    if "nc" not in _CACHE:
        _CACHE["nc"] = _build_nc()
    return _CACHE["nc"]


def kernel(x, cond_emb, ln_gamma, ln_beta, in_proj_w, in_proj_b, out_w, out_b, kv_w, kv_b):
    x = np.asarray(x, dtype=np.float32)
    nc = get_nc()
    in_maps = make_in_maps(
        x,
        np.asarray(cond_emb, np.float32),
        np.asarray(in_proj_w, np.float32),
        np.asarray(in_proj_b, np.float32),
        np.asarray(out_w, np.float32),
        np.asarray(out_b, np.float32),
        np.asarray(kv_w, np.float32),
        np.asarray(kv_b, np.float32),
    )
    res = run_bass_kernel_spmd(nc, in_maps, core_ids=list(range(NCORES)))
    y = np.empty((B, C, H, W), np.float32)
    for r in range(NCORES):
        y[r * BS : (r + 1) * BS] = res.results[r]["y"].reshape(BS, C, H, W)
    return y
